# revision 1
# baseline (speedup 1.0000x reference)
"""Trainium2 Bass kernel for nn_NeuralMemory (B=4, N=1024, D=128, DEPTH=4).

Sharding: 8 cores, core c handles batch b = c//2. The store phase
(per-token grads of the 4 memory weights, summed over the sequence) is
computed redundantly by both cores of a pair -- the grad sum is
permutation invariant over tokens, so each core is fed its batch's
sequence with its own retrieval half rotated to the front and retrieves
tokens [0:512) of its view. No cross-core communication (a pair-wise
AllReduce has a ~10us floor, worse than the duplicated compute).

Layout: activations are feature-major [D=128 partitions, tokens]; the
store phase runs in two 512-token tiles. Layer matmuls are
matmul(out^T, lhsT=W, rhs=X^T) with float32r operands (~2e-4
per-matmul rel err on HW). dW_i = A_i^T @ G_i contracts over tokens,
so A/G get bf16 copies rotated token-major via PE transposes (4 chunks
per PSUM bank); dW matmuls run bf16 with fp32 PSUM accumulation.
dW3/dW2/dW1 and M = S^T @ G0 share one PSUM bank (a single
accumulation group). Tile-1's H tiles borrow the dW-transpose ("tr")
PSUM banks, which idle until mid-kernel, so both tiles' forwards
pipeline; a few dummy matmuls at t=0 hold the PE HAM clock window busy
so the first transposes run at full clock.

K is never materialized: H0 = S @ (Wk @ w0) with the [D,D] composition
on-chip, and the retrieval's first layer is rewritten
  X1 = X0 @ w0 + (X0 @ Wk^T) @ M,   X0 = S @ wq
so X0 and P^T = Wk @ X0^T are computed early and only the tiny
M-eviction sits on the critical tail (U0/dW0 never materialize).
V is folded into H3's PSUM accumulation with a negated Wv (G3 raw =
H3 - V straight out of one bank; the 2/D scale lives in w3^T and in
the a3 bf16 cast).

All weights arrive in ONE packed DRAM tensor (HWDGE dispatch is ~625ns
per dma_start, serialized); seq arrives in 2 halves plus a casting
SWDGE bf16 copy.

ACT-table discipline: all forward Silus before any Derivative_silu
(H0..H2 evicted to SBUF), and a dummy Silu reloads the silu table
during the dW phase so the retrieval tail pays no table load.
"""

import numpy as np

import concourse.bass as bass
import concourse.mybir as mybir
import concourse.tile as tile
from concourse import bacc
from concourse.bass import ts
from concourse.bass_utils import run_bass_kernel_spmd
from concourse.masks import make_identity

B, N, D = 4, 1024, 128
DEPTH = 4
NCORES = 8
NT = 512            # tokens retrieved per core (half a batch)
TT = 512            # store-phase token tile
NTI = N // TT       # store tiles
NCHUNK = N // 128   # 8 token chunks of 128
RH = 256            # retrieval sub-tile
WPACK = 4 * D + D + 2 * D   # w0..w3 | wq | wkv

f32 = mybir.dt.float32
f32r = mybir.dt.float32r
bf16 = mybir.dt.bfloat16

AF = mybir.ActivationFunctionType
ALU = mybir.AluOpType

TM_DT = bf16


def _build_program(reps=1):
    nc = bacc.Bacc(
        "TRN2",
        target_bir_lowering=False,
        debug=False,
        enable_asserts=False,
        num_devices=NCORES,
    )

    seq = nc.dram_tensor("seq", [N, D], f32, kind="ExternalInput").ap()
    wp_dr = nc.dram_tensor("wpack", [D, WPACK], f32, kind="ExternalInput").ap()
    out_dr = nc.dram_tensor("out", [NT, D], f32, kind="ExternalOutput").ap()

    with tile.TileContext(nc) as tc:
        for _ in range(reps):
            _emit(tc, seq, wp_dr, out_dr)

    nc.compile()
    return nc


def _emit(tc, seq, wp_dr, out_dr):
    nc = tc.nc
    from contextlib import ExitStack

    from concourse.tile_rust import add_dep_helper as _dep  # type: ignore

    with ExitStack() as ctx:
        consts = ctx.enter_context(tc.tile_pool(name="consts", bufs=1))
        big = ctx.enter_context(tc.tile_pool(name="big", bufs=1))
        # PSUM banks: mm(2) + hold(2) + tr(3) + dw(1) = 8
        pp = ctx.enter_context(tc.tile_pool(name="pp", bufs=1, space="PSUM"))

        def pmm(name, w=512):
            return pp.tile([128, w], f32, tag="mm", bufs=2, name=name)

        def phold(name, w=512):
            return pp.tile([128, w], f32, tag="hold", bufs=2, name=name)

        def ptr(name):
            return pp.tile([128, 512], TM_DT, tag="tr", bufs=3, name=name)

        # tiny scratch silu pulls the first ACT table load off the
        # critical path (runs during the DMAs)
        scr = consts.tile([128, 1], f32, tag="scr")
        scr2 = consts.tile([128, 1], f32, tag="scr2")
        nc.gpsimd.memset(scr[:], 0.0)
        nc.scalar.activation(scr2[:], scr[:], AF.Silu)

        # PE warm-up: keep the HAM clock window busy before real work so
        # the S^T transposes and first matmuls run at full clock
        wupa = consts.tile([128, 128], f32r, tag="wupa")
        nc.gpsimd.memset(wupa[:].bitcast(f32), 0.0)
        wupp = pp.tile([128, 512], f32, tag="tr", bufs=3, name="wupp")
        for k in range(3):
            nc.tensor.matmul(
                wupp[:, 0:128], wupa[:], wupa[:],
                skip_group_check=True,
            )

        ident = consts.tile([128, 128], f32, tag="ident")
        make_identity(nc, ident)
        ident_b = consts.tile([128, 128], bf16, tag="ident_b")
        nc.gpsimd.tensor_copy(ident_b[:], ident[:])

        # ---- DMAs ordered by need ----
        wp = consts.tile([D, WPACK], f32, tag="wp")
        nc.sync.dma_start(wp[:], wp_dr)
        w_sb = [wp[:, ts(i, D)] for i in range(4)]
        wq_sb = wp[:, ts(4, D)]
        wkv_sb = wp[:, 5 * D : 7 * D]

        s_tm = big.tile([128, NCHUNK, 128], f32, tag="s_tm")
        seq_r = seq.rearrange("(c p) d -> p c d", p=128)
        nc.sync.dma_start(s_tm[:, 0:4], seq_r[:, 0:4])
        nc.sync.dma_start(s_tm[:, 4:8], seq_r[:, 4:8])
        s_tmb = big.tile([128, NCHUNK, 128], bf16, tag="s_tmb")

        # persistent SBUF activations (feature-major)
        st = big.tile([128, N], f32r, tag="st")
        a1 = big.tile([128, N], f32r, tag="a1")
        a2 = big.tile([128, N], f32r, tag="a2")
        a3 = big.tile([128, N], f32r, tag="a3")
        hsb = big.tile([128, 3, N], f32, tag="hsb")     # H0..H2 in SBUF
        sp0 = big.tile([128, N], f32, tag="sp0")
        sp1 = big.tile([128, N], f32, tag="sp1")
        sp2 = big.tile([128, N], f32, tag="sp2")
        g1 = big.tile([128, N], f32r, tag="g1")
        g2 = big.tile([128, N], f32r, tag="g2")
        g3 = big.tile([128, N], f32r, tag="g3")         # raw H3 - V
        # bf16 copies for the dW path (a3b carries the 2/D scale)
        a1b = big.tile([128, N], TM_DT, tag="a1b")
        a2b = big.tile([128, N], TM_DT, tag="a2b")
        a3b = big.tile([128, N], TM_DT, tag="a3b")
        g0b = big.tile([128, N], TM_DT, tag="g0b")
        g1b = big.tile([128, N], TM_DT, tag="g1b")
        g2b = big.tile([128, N], TM_DT, tag="g2b")
        g3b = big.tile([128, N], TM_DT, tag="g3b")

        wt = big.tile([128, 3, 128], f32r, tag="wt")    # w1^T,w2^T,w3^T*(2/D)
        wk_t = big.tile([128, 128], f32, tag="wk_t")    # Wk^T (fp32)
        wk_tr = big.tile([128, 128], f32r, tag="wk_tr")  # Wk^T (f32r)
        w0eff = big.tile([128, 128], f32r, tag="w0eff")  # Wk @ w0
        w0r = big.tile([128, 128], f32r, tag="w0r")
        wqr = big.tile([128, 128], f32r, tag="wqr")
        wv_r = big.tile([D, D], f32r, tag="wv_r")       # -Wv
        w_r = [None] + [
            big.tile([D, D], f32r, name=f"wr{i}", tag=f"wr{i}") for i in (1, 2, 3)
        ]
        for i in (1, 2, 3):
            nc.vector.tensor_copy(w_r[i][:], w_sb[i])
        # negated so V accumulates as -V into H3's PSUM bank
        nc.vector.tensor_scalar_mul(wv_r[:], wkv_sb[:, D : 2 * D], -1.0)
        nc.vector.tensor_copy(w0r[:], w_sb[0])
        nc.vector.tensor_copy(wqr[:], wq_sb)

        silu_insts = []
        dsilu_insts = []

        # ---- S^T (before weight-gated work: pool slots stay free) (fp32 PE transposes, evictions round to f32r) ----
        for g in range(NCHUNK // 4):
            p = pmm(f"p_st{g}")
            for j in range(4):
                nc.tensor.transpose(p[:, ts(j, 128)], s_tm[:, g * 4 + j], ident)
            nc.vector.tensor_copy(st[:, ts(g, 512)], p[:])

        # ---- setup transposes + W0eff ----
        p = pmm("p_tr1")
        nc.tensor.transpose(p[:, ts(0, 128)], wkv_sb[:, 0:D], ident)
        for i in range(2):
            nc.tensor.transpose(p[:, ts(1 + i, 128)], w_sb[1 + i], ident)
        nc.tensor.transpose(p[:, ts(3, 128)], w_sb[3], ident)
        nc.vector.tensor_copy(wk_t[:], p[:, 0:128])
        nc.vector.tensor_copy(wk_tr[:], p[:, 0:128])
        nc.vector.tensor_copy(
            wt[:, 0:2], p[:, 128:384].rearrange("p (c d) -> p c d", d=128)
        )
        nc.scalar.activation(wt[:, 2], p[:, 384:512], AF.Copy, scale=2.0 / D)

        p = pmm("p_w0eff")
        nc.tensor.matmul(p[:, 0:128], wk_t[:], w_sb[0])
        nc.vector.tensor_copy(w0eff[:], p[:, 0:128])

        # ---- X0^T = wq^T S^T and P^T = Wk X0^T (ACT evictions: DVE is the
        # fwd-setup bottleneck and ACT idles until the first Silu) ----
        x0 = big.tile([128, NT], f32r, tag="x0")
        px = pmm("p_x0")
        nc.tensor.matmul(px[:], wqr[:], st[:, 0:NT])
        nc.vector.tensor_copy(x0[:], px[:])
        pt = big.tile([128, NT], f32r, tag="pt")
        px = pmm("p_pt")
        nc.tensor.matmul(px[:], wk_tr[:], x0[:])
        nc.vector.tensor_copy(pt[:], px[:])

        # ---- forward: all Silus first; H2 held in PSUM, H0/H1 to SBUF ----
        holds = {}
        for t in range(NTI):
            sl = ts(t, TT)
            hloc = []
            for li in range(3):
                wst = (w0eff, w_r[1], w_r[2])[li]
                rhs = (st, a1, a2)[li]
                if li == 2:
                    h = phold(f"h{li}_{t}", TT)
                elif t == 1:
                    # tile-1 H0/H1 borrow the (idle until dW) tr banks
                    h = pp.tile([128, TT], f32, tag="tr", bufs=3, name=f"h{li}_{t}")
                else:
                    h = pmm(f"h{li}_{t}", TT)
                nc.tensor.matmul(h[:], wst[:], rhs[:, sl])
                dst = (a1, a2, a3)[li]
                silu_insts.append(nc.scalar.activation(dst[:, sl], h[:], AF.Silu))
                if li == 2:
                    hloc.append(h[:])
                else:
                    nc.vector.tensor_copy(hsb[:, li, sl], h[:])
                    hloc.append(hsb[:, li, sl])
            # H3 - V accumulated in one PSUM bank (wv_r is negated)
            if t == 1:
                h3 = pp.tile([128, TT], f32, tag="tr", bufs=3, name=f"h3_{t}")
            else:
                h3 = pmm(f"h3_{t}", TT)
            nc.tensor.matmul(h3[:], w_r[3][:], a3[:, sl], start=True, stop=False)
            nc.tensor.matmul(h3[:], wv_r[:], st[:, sl], start=False, stop=True)
            nc.vector.tensor_copy(g3[:, sl], h3[:])     # raw H3 - V
            nc.gpsimd.tensor_copy(a1b[:, sl], a1[:, sl].bitcast(f32))
            nc.gpsimd.tensor_copy(a2b[:, sl], a2[:, sl].bitcast(f32))
            nc.gpsimd.tensor_scalar_mul(
                a3b[:, sl], a3[:, sl].bitcast(f32), 2.0 / D
            )
            nc.gpsimd.tensor_copy(g3b[:, sl], g3[:, sl].bitcast(f32))
            holds[t] = hloc

        # ---- backward: Derivative_silu after all Silus + chains ----
        for t in range(NTI):
            di = nc.scalar.activation(
                sp2[:, ts(t, TT)], holds[t][2], AF.Derivative_silu
            )
            dsilu_insts.append(di)
        dsilu_insts.append(
            nc.scalar.activation(sp1[:], hsb[:, 1, :], AF.Derivative_silu)
        )
        dsilu_insts.append(
            nc.scalar.activation(sp0[:], hsb[:, 0, :], AF.Derivative_silu)
        )
        for t in range(NTI):
            sl = ts(t, TT)

            c2 = pmm(f"c2_{t}", TT)
            nc.tensor.matmul(c2[:], wt[:, 2], g3[:, sl])
            nc.vector.tensor_mul(g2[:, sl], c2[:], sp2[:, sl])

            c1 = pmm(f"c1_{t}", TT)
            nc.tensor.matmul(c1[:], wt[:, 1], g2[:, sl])
            nc.vector.tensor_mul(g1[:, sl], c1[:], sp1[:, sl])

            c0 = pmm(f"c0_{t}", TT)
            nc.tensor.matmul(c0[:], wt[:, 0], g1[:, sl])
            nc.vector.tensor_mul(g0b[:, sl], c0[:], sp0[:, sl])  # bf16 direct
            nc.gpsimd.tensor_copy(g2b[:, sl], g2[:, sl].bitcast(f32))
            nc.gpsimd.tensor_copy(g1b[:, sl], g1[:, sl].bitcast(f32))

        for di in dsilu_insts:
            _dep(di.ins, silu_insts[-1].ins, sync=False, reason="act-table order")

        # bf16 seq copy for the M matmuls -- held back (dep on the first
        # Silu) so its transfer doesn't delay the seq/weight DMAs at startup
        _stmb_dma = nc.gpsimd.dma_start(s_tmb[:], seq_r)
        _dep(_stmb_dma.ins, silu_insts[0].ins, sync=False,
             reason="defer bf16 seq copy off the startup DMA path")

        # ---- token-major transposes + dW accumulation ---------------------
        a_tm = [None] + [
            big.tile([128, N], TM_DT, name=f"atm{i}", tag=f"atm{i}") for i in (1, 2, 3)
        ]
        g_tm = [
            big.tile([128, N], TM_DT, name=f"gtm{i}", tag=f"gtm{i}") for i in range(4)
        ]
        u = [
            None,
            consts.tile([D, D], f32r, name="u1", tag="u1"),
            consts.tile([D, D], f32r, name="u2", tag="u2"),
            consts.tile([D, D], f32r, name="u3", tag="u3"),
        ]

        # reload the silu table during the dW phase, off the tail
        scr3 = consts.tile([128, 1], f32, tag="scr3")
        dummy = nc.scalar.activation(scr3[:], scr[:], AF.Silu)
        _dep(dummy.ins, dsilu_insts[-1].ins, sync=False, reason="act-table order")

        evict_flip = [0]

        def transpose_half(src, dst, h, name):
            p = ptr(name)
            for j in range(4):
                c = h * 4 + j
                nc.tensor.matmul(
                    p[:, ts(j, 128)], src[:, ts(c, 128)], ident_b[:],
                    is_transpose=True,
                )
            if evict_flip[0] % 3 == 2:
                nc.scalar.activation(dst[:, ts(h, 512)], p[:], AF.Copy)
            else:
                nc.vector.tensor_copy(dst[:, ts(h, 512)], p[:])
            evict_flip[0] += 1

        # dW3/dW2/dW1 and M share one PSUM bank (one accumulation group)
        acc = pp.tile([128, 4, 128], f32, tag="dw", bufs=1, name="dwacc")
        first = [True]

        for i, (ab, gb, atm, gtm, slot) in enumerate(
            (
                (a3b, g3b, a_tm[3], g_tm[3], 0),
                (a2b, g2b, a_tm[2], g_tm[2], 1),
                (a1b, g1b, a_tm[1], g_tm[1], 2),
            )
        ):
            for h in range(2):
                transpose_half(ab, atm, h, f"p_a{i}{h}")
                transpose_half(gb, gtm, h, f"p_g{i}{h}")
                for j in range(4):
                    c = h * 4 + j
                    nc.tensor.matmul(
                        acc[:, slot],
                        atm[:, ts(c, 128)],
                        gtm[:, ts(c, 128)],
                        start=first[0],
                        stop=False,
                    )
                    first[0] = False

        # M = S^T @ G0 into acc slot 3 (last writes of the bank group)
        for h in range(2):
            transpose_half(g0b, g_tm[0], h, f"p_g0{h}")
            for j in range(4):
                c = h * 4 + j
                nc.tensor.matmul(
                    acc[:, 3],
                    s_tmb[:, c],
                    g_tm[0][:, ts(c, 128)],
                    start=False,
                    stop=(h == 1 and j == 3),
                )
        m_r = big.tile([128, 128], f32r, tag="m_r")
        nc.vector.tensor_copy(m_r[:], acc[:, 3])
        for slot, i in ((2, 1), (1, 2), (0, 3)):
            nc.vector.tensor_add(u[i][:], acc[:, slot], w_sb[i])

        # ---- retrieval: X1 = X0 @ w0 + P @ M, then layers 2..4 ------------
        r1 = big.tile([128, NT], f32r, tag="r1")
        r2 = big.tile([128, NT], f32r, tag="r2")
        r3 = big.tile([128, NT], f32r, tag="r3")
        o_tm = big.tile([128, NT // 128, 128], f32, tag="o_tm")
        out_r = out_dr.rearrange("(c p) d -> p c d", p=128)

        nh = NT // RH
        px1s = []
        for hh in range(nh):
            sl = ts(hh, RH)
            px = phold(f"px1_{hh}", RH)
            # term 1 (X0 @ w0) has no M dependency -- runs during the dW phase
            nc.tensor.matmul(px[:], w0r[:], x0[:, sl], start=True, stop=False)
            px1s.append(px)
        for hh in range(nh):
            sl = ts(hh, RH)
            px = px1s[hh]
            nc.tensor.matmul(px[:], m_r[:], pt[:, sl], start=False, stop=True)
            nc.scalar.activation(r1[:, sl], px[:], AF.Silu)
        for hh in range(nh):
            sl = ts(hh, RH)
            px = phold(f"px2_{hh}", RH)
            nc.tensor.matmul(px[:], u[1][:], r1[:, sl])
            nc.scalar.activation(r2[:, sl], px[:], AF.Silu)
        for hh in range(nh):
            sl = ts(hh, RH)
            px = pmm(f"px3_{hh}", RH)
            nc.tensor.matmul(px[:], u[2][:], r2[:, sl])
            nc.scalar.activation(r3[:, sl], px[:], AF.Silu)
        for hh in range(nh):
            po = pmm(f"po{hh}", RH)
            for j in range(RH // 128):
                c = hh * (RH // 128) + j
                nc.tensor.matmul(
                    po[:, ts(j, 128)],
                    r3[:, ts(c, 128)],
                    u[3][:],
                    start=(j == 0),
                    stop=(j == RH // 128 - 1),
                )
            nc.vector.tensor_copy(
                o_tm[:, 2 * hh : 2 * hh + 2],
                po[:].rearrange("p (c d) -> p c d", d=128),
            )
            nc.sync.dma_start(
                out_r[:, 2 * hh : 2 * hh + 2], o_tm[:, 2 * hh : 2 * hh + 2]
            )


_CACHE = {}


def _get_nc():
    if "nc" not in _CACHE:
        _CACHE["nc"] = _build_program()
    return _CACHE["nc"]


def kernel(seq, w0, w1, w2, w3, wq, wkv):
    nc = _get_nc()
    seq = np.ascontiguousarray(np.asarray(seq, np.float32))
    wpack = np.ascontiguousarray(
        np.concatenate(
            [np.asarray(x, np.float32) for x in (w0, w1, w2, w3, wq, wkv)], axis=1
        )
    )

    in_maps = []
    for c in range(NCORES):
        b, h = c // 2, c % 2
        if h == 0:
            s = seq[b]
        else:
            # rotate: retrieval half first; grad sum is order-invariant
            s = np.concatenate([seq[b, NT:], seq[b, :NT]], axis=0)
        in_maps.append({"seq": np.ascontiguousarray(s), "wpack": wpack})

    res = run_bass_kernel_spmd(nc, in_maps, core_ids=list(range(NCORES)))
    _CACHE["last_results"] = res

    out = np.empty((B, N, D), np.float32)
    for c in range(NCORES):
        b, h = c // 2, c % 2
        out[b, h * NT : (h + 1) * NT] = res.results[c]["out"]
    return out



# revision 19
# speedup vs baseline: 1.3452x; 1.3452x over previous
"""Trainium2 Bass kernel for nn_NeuralMemory (B=4, N=1024, D=128, DEPTH=4).

Sharding: 8 cores, core c handles batch b = c//2. Both cores of a pair
compute the store phase (per-token grads summed over all 1024 tokens)
redundantly -- the grad sum is order-invariant, so each core gets its
batch's sequence with its own retrieval half rotated to the front and
retrieves tokens [0:512) of its view. No collectives (a pair AllReduce
has a ~10us floor, worse than the duplicated compute).

v4 design notes:
  - All on-chip tensors are bf16 except f32 PSUM accumulators, so every
    producer writes bf16 directly (no cast chain).
  - Host-side prep (layout/weight-space only, no token-dim compute):
    seq is shipped twice in bf16 -- token-major (s_tmb, for M = S^T G0)
    and feature-major (st = S^T, for the forward); the [d,d] weights are
    shipped pre-transposed/pre-scaled/pre-composed in one bf16 pack
    (Wk@w0, w^T's, +-(2/D) scales, identity) plus a small f32 pack for
    the u_i = w_i + dW_i adds.
  - Dependency tracking is tile-granular, so every per-half-written or
    per-half-read tensor is split into separate tiles (a_i, g_i, sp_i,
    c_i, H_i, px_i, r_i, o_tm halves) -- otherwise write-after-read
    false deps serialize the two pipelines.
  - forward Silu / backward Derivative_silu read the f32 H PSUM banks
    directly; H0..H2 live in six single-bank tiles whose banks are
    reused by the backward c tiles, then the retrieval px tiles.
  - token-major copies for the dW contraction run on the DMA XBAR
    (dma_start_transpose, SBUF->SBUF bf16, ~450ns per [128,512] half on
    otherwise-idle DMA engines); only g0 (which gates M on the critical
    tail) keeps the lower-latency PE-transpose + DVE-evict path.
  - backward runs tile-1-first layer-major; dW3/dW2/dW1 + M share one
    PSUM accumulation group; retrieval is X1 = X0 w0 + (X0 Wk^T) M with
    the X0 w0 term pre-accumulated, so only the tiny M eviction sits on
    the critical tail.
  - ACT-table discipline: all Silus, then all Derivative_silus, then a
    dummy Silu reload during the dW phase.
"""

import numpy as np
import ml_dtypes

import concourse.bass as bass
import concourse.mybir as mybir
import concourse.tile as tile
from concourse import bacc
from concourse.bass import ts
from concourse.bass_utils import run_bass_kernel_spmd

B, N, D = 4, 1024, 128
NCORES = 8
NT = 512            # tokens retrieved per core (half a batch)
TT = 512            # store-phase token tile
NTI = N // TT
NCHUNK = N // 128
RH = 256            # retrieval sub-tile

# bf16 weight packs:
#  wpbu (urgent): w0eff=Wk@w0 | w1 | w2 | w3s=(2/D)w3 | wv_r=-(2/D)Wv
#                 | wq | wkq_t=wq@Wk^T
#  wpbr (rest):   w1^T | w2^T | w3^T | w0 | ident

f32 = mybir.dt.float32
bf16 = mybir.dt.bfloat16

AF = mybir.ActivationFunctionType
ALU = mybir.AluOpType


def _build_program(reps=1):
    nc = bacc.Bacc(
        "TRN2",
        target_bir_lowering=False,
        debug=False,
        enable_asserts=False,
        num_devices=NCORES,
    )

    st_dr = nc.dram_tensor("st", [128, N], bf16, kind="ExternalInput").ap()
    stm_dr = nc.dram_tensor("s_tmb", [128, N], bf16, kind="ExternalInput").ap()
    we_dr = nc.dram_tensor("w0eff", [D, D], bf16, kind="ExternalInput").ap()
    wbu_dr = nc.dram_tensor("wpbu", [D, 6 * D], bf16, kind="ExternalInput").ap()
    wbr_dr = nc.dram_tensor("wpbr", [D, 5 * D], bf16, kind="ExternalInput").ap()
    wf_dr = nc.dram_tensor("wpf", [D, 3 * D], f32, kind="ExternalInput").ap()
    out_dr = nc.dram_tensor("out", [NT, D], bf16, kind="ExternalOutput").ap()

    with tile.TileContext(nc) as tc:
        for _ in range(reps):
            _emit(tc, st_dr, stm_dr, we_dr, wbu_dr, wbr_dr, wf_dr, out_dr)

    nc.compile()
    return nc


def _emit(tc, st_dr, stm_dr, we_dr, wbu_dr, wbr_dr, wf_dr, out_dr):
    nc = tc.nc
    from contextlib import ExitStack

    from concourse.tile_rust import add_dep_helper as _dep  # type: ignore

    with ExitStack() as ctx:
        consts = ctx.enter_context(tc.tile_pool(name="consts", bufs=1))
        big = ctx.enter_context(tc.tile_pool(name="big", bufs=1))
        # PSUM: ha0,ha1,hb0,hb1,hc0,hc1 (1 bank each) + st(2) = 8 banks
        pp = ctx.enter_context(tc.tile_pool(name="pp", bufs=1, space="PSUM"))

        def pbank(tag, name, shape=None, dt=f32):
            return pp.tile(shape or [128, TT], dt, tag=tag, bufs=1, name=name)

        def pstage(name, w=512, dt=f32):
            return pp.tile([128, w], dt, tag="stg", bufs=2, name=name)

        # ---- DMAs, ordered by need (all HWDGE on the sync queue);
        # w0eff ships alone (32KB) so layer 0 starts right after S^T ----
        w0eff_t = consts.tile([D, D], bf16, tag="w0eff")
        wpbu = consts.tile([D, 6 * D], bf16, tag="wpbu")
        wpbr = consts.tile([D, 5 * D], bf16, tag="wpbr")
        stt = big.tile([128, N], bf16, tag="stt")      # S^T feature-major
        s_tmb = big.tile([128, NCHUNK, 128], bf16, tag="s_tmb")  # token-major
        wpf = consts.tile([D, 3 * D], f32, tag="wpf")
        nc.sync.dma_start(w0eff_t[:], we_dr)
        nc.sync.dma_start(stt[:], st_dr)
        nc.sync.dma_start(wpbu[:], wbu_dr)
        nc.sync.dma_start(wpbr[:], wbr_dr)
        nc.sync.dma_start(
            s_tmb[:], stm_dr.rearrange("p (c d) -> p c d", d=128)
        )
        nc.sync.dma_start(wpf[:], wf_dr)

        w0eff = w0eff_t[:]
        w1b = wpbu[:, ts(0, D)]
        w2b = wpbu[:, ts(1, D)]
        w3s = wpbu[:, ts(2, D)]
        wv_r = wpbu[:, ts(3, D)]
        wqb = wpbu[:, ts(4, D)]
        wkq_t = wpbu[:, ts(5, D)]                      # wq @ Wk^T
        wt = [wpbr[:, ts(i, D)] for i in range(3)]     # w1^T,w2^T,w3^T
        w0b = wpbr[:, ts(3, D)]
        ident_b = wpbr[:, ts(4, D)]
        w_f = [wpf[:, ts(i, D)] for i in range(3)]     # w1,w2,w3 f32

        # tiny scratch silu pulls the first ACT table load off the
        # critical path (runs during the DMAs)
        scr = consts.tile([128, 1], f32, tag="scr")
        scr2 = consts.tile([128, 1], f32, tag="scr2")
        nc.gpsimd.memset(scr[:], 0.0)
        nc.scalar.activation(scr2[:], scr[:], AF.Silu)

        # PE warm-up: start the HAM clock window early so real matmuls
        # hit full clock by ~3us
        wupa = consts.tile([128, 128], f32, tag="wupa")
        nc.gpsimd.memset(wupa[:], 0.0)
        wupp = pstage("wupp")
        for _ in range(6):
            nc.tensor.matmul(
                wupp[:, 0:128], wupa[:], wupa[:],
                skip_group_check=True,
            )

        # per-half persistent SBUF tensors (feature-major, bf16)
        def halves(pfx):
            return [
                big.tile([128, TT], bf16, name=f"{pfx}{t}", tag=f"{pfx}{t}")
                for t in range(NTI)
            ]

        a1 = halves("a1")
        a2 = halves("a2")
        a3 = halves("a3")
        sp0 = halves("sp0")
        sp1 = halves("sp1")
        sp2 = halves("sp2")
        g0 = halves("g0")
        g1 = halves("g1")
        g2 = halves("g2")
        g3 = halves("g3")   # (2/D)(H3 - V)

        # ---- x0 = wq^T S^T and pt = (wq Wk^T)^T S^T: two independent
        # matmuls straight off S^T (host pre-composed wkq_t = wq @ Wk^T),
        # evicted on DVE long before the backward muls need it ----
        x0 = big.tile([128, NT], bf16, tag="x0")
        px = pstage("p_x0")
        nc.tensor.matmul(px[:], wqb, stt[:, 0:NT])
        nc.vector.tensor_copy(x0[:], px[:])
        pt = big.tile([128, NT], bf16, tag="pt")
        px = pstage("p_pt")
        nc.tensor.matmul(px[:], wkq_t, stt[:, 0:NT])
        nc.vector.tensor_copy(pt[:], px[:])

        silu_insts = []
        dsilu_insts = []

        # ---- forward: six single-bank H tiles; silu -> bf16 halves ----
        hb = {}
        for li, tag in ((0, "ha"), (1, "hb"), (2, "hc")):
            for t in range(NTI):
                hb[li, t] = pbank(f"{tag}{t}", f"h{li}_{t}")
        for t in range(NTI):
            sl = ts(t, TT)
            nc.tensor.matmul(hb[0, t][:], w0eff, stt[:, sl])
            silu_insts.append(nc.scalar.activation(a1[t][:], hb[0, t][:], AF.Silu))
            nc.tensor.matmul(hb[1, t][:], w1b, a1[t][:])
            silu_insts.append(nc.scalar.activation(a2[t][:], hb[1, t][:], AF.Silu))
            nc.tensor.matmul(hb[2, t][:], w2b, a2[t][:])
            silu_insts.append(nc.scalar.activation(a3[t][:], hb[2, t][:], AF.Silu))
            # H3 - V accumulated in one stage bank (wv_r is negated+scaled)
            h3 = pstage(f"h3_{t}")
            nc.tensor.matmul(h3[:], wv_r, stt[:, sl], start=True, stop=False)
            nc.tensor.matmul(h3[:], w3s, a3[t][:], start=False, stop=True)
            nc.vector.tensor_copy(g3[t][:], h3[:])

        # ---- dsilu after all silus (one table switch), consumption order --
        for spd, li in ((sp2, 2), (sp1, 1), (sp0, 0)):
            for t in (1, 0):
                di = nc.scalar.activation(
                    spd[t][:], hb[li, t][:], AF.Derivative_silu
                )
                dsilu_insts.append(di)
        for di in dsilu_insts:
            _dep(di.ins, silu_insts[-1].ins, sync=False, reason="act-table order")

        # ---- backward: tile-1 first (it gates M); c tiles reuse the H
        # banks (freed in dsilu order, which matches mul consumption) ----
        cb = {}
        for li, tag in ((2, "hc"), (1, "hb"), (0, "ha")):
            for t in (1, 0):
                cb[li, t] = pbank(f"{tag}{t}", f"c{li}_{t}")
        for li, gin, gout, spd in (
            (2, g3, g2, sp2), (1, g2, g1, sp1), (0, g1, g0, sp0)
        ):
            for t in (1, 0):
                nc.tensor.matmul(cb[li, t][:], wt[li], gin[t][:])
                nc.vector.tensor_mul(gout[t][:], cb[li, t][:], spd[t][:])

        # reload the silu table during the dW phase, off the tail
        scr3 = consts.tile([128, 1], f32, tag="scr3")
        dummy = nc.scalar.activation(scr3[:], scr[:], AF.Silu)
        _dep(dummy.ins, dsilu_insts[-1].ins, sync=False, reason="act-table order")

        # ---- token-major copies: XBAR for a1..a3,g3..g1; PE path for g0 ----
        a_tm = [None] + [
            big.tile([128, NCHUNK, 128], bf16, name=f"atm{i}", tag=f"atm{i}")
            for i in (1, 2, 3)
        ]
        g_tm = [
            big.tile([128, NCHUNK, 128], bf16, name=f"gtm{i}", tag=f"gtm{i}")
            for i in range(4)
        ]
        for src, dst in (
            (a1, a_tm[1]), (a2, a_tm[2]), (a3, a_tm[3]),
            (g3, g_tm[3]), (g2, g_tm[2]), (g1, g_tm[1]),
        ):
            for h in range(NTI):
                nc.sync.dma_start_transpose(dst[:, 4 * h : 4 * h + 4], src[h][:])

        # g0: PE transposes + DVE half evicts (lowest latency on the
        # tail); separate stage tiles per half so the h0 transposes don't
        # wait on the h1 eviction (tile-granular WAR). Both g0 muls are
        # emitted before the evicts so DVE drains the muls first.
        pg0 = [pstage(f"p_g0{h}", w=512, dt=bf16) for h in range(NTI)]
        for h in (1, 0):
            for j in range(4):
                nc.tensor.matmul(
                    pg0[h][:, ts(j, 128)], g0[h][:, ts(j, 128)], ident_b,
                    is_transpose=True,
                )
        for h in (1, 0):
            nc.vector.tensor_copy(
                g_tm[0][:, 4 * h : 4 * h + 4],
                pg0[h][:].rearrange("p (c d) -> p c d", d=128),
            )

        # ---- M = S^T G0 in its OWN bank/group so m_r never waits the
        # late dW1 xbar round-trip ----
        macc = pstage("macc", w=128)
        m_stop = None
        for k, c in enumerate((4, 5, 6, 7, 0, 1, 2, 3)):
            m_stop = nc.tensor.matmul(
                macc[:, 0:128],
                s_tmb[:, c],
                g_tm[0][:, c],
                start=(k == 0),
                stop=(c == 3),
            )
        m_r = big.tile([128, 128], bf16, tag="m_r")
        nc.vector.tensor_copy(m_r[:], macc[:, 0:128])

        # ---- dW3/dW2/dW1 in one PSUM accumulation group ----
        acc = pbank("hc1", "dwacc", shape=[128, 3, 128])
        dw_stop = None
        for k, (i, slot) in enumerate(((3, 0), (2, 1), (1, 2))):
            for c in range(NCHUNK):
                dw_stop = nc.tensor.matmul(
                    acc[:, slot],
                    a_tm[i][:, c],
                    g_tm[i][:, c],
                    start=(k == 0 and c == 0),
                    stop=(slot == 2 and c == NCHUNK - 1),
                )

        u = [None]
        for slot, i in ((2, 0), (1, 1), (0, 2)):
            ut = big.tile([D, D], bf16, name=f"u{i}", tag=f"u{i}")
            ai = nc.vector.tensor_add(ut[:], acc[:, slot], w_f[i])
            # same-bank safety: no reads before the group's stop matmul
            _dep(ai.ins, dw_stop.ins, sync=True, reason="acc bank group")
            u.append(ut)
        # u[1]=w1+dW1, u[2]=w2+dW2, u[3]=w3+dW3

        # ---- retrieval: X1 = X0 @ w0 + P @ M, then layers 2..4 ------------
        # per-half tiles throughout so the two half-pipelines don't
        # serialize on tile-granular deps
        r1, r2, r3 = [], [], []
        for h in range(NTI):
            r1.append(big.tile([128, RH], bf16, name=f"r1h{h}", tag=f"r1h{h}"))
            r2.append(big.tile([128, RH], bf16, name=f"r2h{h}", tag=f"r2h{h}"))
            r3.append(big.tile([128, RH], bf16, name=f"r3h{h}", tag=f"r3h{h}"))

        nh = NT // RH
        px1 = [pbank(f"ha{hh}", f"px1_{hh}", shape=[128, RH]) for hh in range(nh)]
        for hh in range(nh):
            # term 1 (X0 @ w0) has no M dependency -- runs during the dW phase
            nc.tensor.matmul(
                px1[hh][:], w0b, x0[:, ts(hh, RH)], start=True, stop=False
            )
        for hh in range(nh):
            nc.tensor.matmul(
                px1[hh][:], m_r[:], pt[:, ts(hh, RH)], start=False, stop=True
            )
            nc.scalar.activation(r1[hh][:], px1[hh][:], AF.Silu)
        px2 = [pbank(f"hb{hh}", f"px2_{hh}", shape=[128, RH]) for hh in range(nh)]
        for hh in range(nh):
            nc.tensor.matmul(px2[hh][:], u[1][:], r1[hh][:])
            nc.scalar.activation(r2[hh][:], px2[hh][:], AF.Silu)
        px3 = [
            pbank("hc0", "px3_0", shape=[128, RH]),
            pstage("px3_1", w=RH),
        ]
        for hh in range(nh):
            nc.tensor.matmul(px3[hh][:], u[2][:], r2[hh][:])
            nc.scalar.activation(r3[hh][:], px3[hh][:], AF.Silu)
        out_r = out_dr.rearrange("(c p) d -> p c d", p=128)
        o_tm = big.tile([128, 4, 128], bf16, tag="o_tm")
        for hh in range(nh):
            po = pstage(f"po{hh}", w=RH)
            pov = po[:].rearrange("p (c d) -> p c d", d=128)
            for j in range(RH // 128):
                nc.tensor.matmul(
                    pov[:, j], r3[hh][:, ts(j, 128)], u[3][:],
                    start=(j == 0), stop=(j == RH // 128 - 1),
                )
            nc.vector.tensor_copy(o_tm[:, 2 * hh : 2 * hh + 2], pov[:])
        # one out DMA: a second HWDGE gen would serialize +625ns on the tail
        nc.sync.dma_start(out_r[:], o_tm[:])


_CACHE = {}


def _get_nc():
    if "nc" not in _CACHE:
        _CACHE["nc"] = _build_program()
    return _CACHE["nc"]


def _bf(x):
    return np.ascontiguousarray(x.astype(ml_dtypes.bfloat16))


def _prep_weights(w0, w1, w2, w3, wq, wkv):
    """Host-side weight-space prep (layout, transposes, scales, composes)."""
    w0, w1, w2, w3, wq, wkv = (
        np.asarray(x, np.float32) for x in (w0, w1, w2, w3, wq, wkv)
    )
    wk, wv = wkv[:, :D], wkv[:, D:]
    ident = np.eye(D, dtype=np.float32)
    w0eff = wk @ w0
    wpbu = np.concatenate(
        [
            w1, w2,
            (2.0 / D) * w3,     # w3s
            (-2.0 / D) * wv,    # wv_r
            wq,                 # wqb
            wq @ wk.T,          # wkq_t: pt = (wq Wk^T)^T S^T
        ],
        axis=1,
    )
    wpbr = np.concatenate([w1.T, w2.T, w3.T, w0, ident], axis=1)
    wpf = np.ascontiguousarray(np.concatenate([w1, w2, w3], axis=1))
    return _bf(w0eff), _bf(wpbu), _bf(wpbr), wpf


def kernel(seq, w0, w1, w2, w3, wq, wkv):
    nc = _get_nc()
    seq = np.asarray(seq, np.float32)
    w0eff, wpbu, wpbr, wpf = _prep_weights(w0, w1, w2, w3, wq, wkv)

    in_maps = []
    for c in range(NCORES):
        b, h = c // 2, c % 2
        if h == 0:
            s = seq[b]
        else:
            # rotate: retrieval half first; grad sum is order-invariant
            s = np.concatenate([seq[b, NT:], seq[b, :NT]], axis=0)
        sb = s.astype(ml_dtypes.bfloat16)
        # token-major [128, c, d] flattened: partition p, token c*128+p
        stm = np.ascontiguousarray(
            sb.reshape(NCHUNK, 128, D).transpose(1, 0, 2).reshape(128, N)
        )
        in_maps.append(
            {
                "st": np.ascontiguousarray(sb.T),
                "s_tmb": stm,
                "w0eff": w0eff,
                "wpbu": wpbu,
                "wpbr": wpbr,
                "wpf": wpf,
            }
        )

    res = run_bass_kernel_spmd(nc, in_maps, core_ids=list(range(NCORES)))
    _CACHE["last_results"] = res

    out = np.empty((B, N, D), np.float32)
    for c in range(NCORES):
        b, h = c // 2, c % 2
        out[b, h * NT : (h + 1) * NT] = res.results[c]["out"].astype(np.float32)
    return out


# revision 22
# speedup vs baseline: 1.3462x; 1.0007x over previous
"""Trainium2 Bass kernel for nn_NeuralMemory (B=4, N=1024, D=128, DEPTH=4).

Sharding: 8 cores, core c handles batch b = c//2. Both cores of a pair
compute the store phase (per-token grads summed over all 1024 tokens)
redundantly -- the grad sum is order-invariant, so each core gets its
batch's sequence with its own retrieval half rotated to the front and
retrieves tokens [0:512) of its view. No collectives (a pair AllReduce
has a ~10us floor, worse than the duplicated compute).

v4 design notes:
  - All on-chip tensors are bf16 except f32 PSUM accumulators, so every
    producer writes bf16 directly (no cast chain).
  - Host-side prep (layout/weight-space only, no token-dim compute):
    seq is shipped twice in bf16 -- token-major (s_tmb, for M = S^T G0)
    and feature-major (st = S^T, for the forward); the [d,d] weights are
    shipped pre-transposed/pre-scaled/pre-composed in one bf16 pack
    (Wk@w0, w^T's, +-(2/D) scales, identity) plus a small f32 pack for
    the u_i = w_i + dW_i adds.
  - Dependency tracking is tile-granular, so every per-half-written or
    per-half-read tensor is split into separate tiles (a_i, g_i, sp_i,
    c_i, H_i, px_i, r_i, o_tm halves) -- otherwise write-after-read
    false deps serialize the two pipelines.
  - forward Silu / backward Derivative_silu read the f32 H PSUM banks
    directly; H0..H2 live in six single-bank tiles whose banks are
    reused by the backward c tiles, then the retrieval px tiles.
  - token-major copies for the dW contraction run on the DMA XBAR
    (dma_start_transpose, SBUF->SBUF bf16, ~450ns per [128,512] half on
    otherwise-idle DMA engines); only g0 (which gates M on the critical
    tail) keeps the lower-latency PE-transpose + DVE-evict path.
  - backward runs tile-1-first layer-major; dW3/dW2/dW1 + M share one
    PSUM accumulation group; retrieval is X1 = X0 w0 + (X0 Wk^T) M with
    the X0 w0 term pre-accumulated, so only the tiny M eviction sits on
    the critical tail.
  - ACT-table discipline: all Silus, then all Derivative_silus, then a
    dummy Silu reload during the dW phase.
"""

import numpy as np
import ml_dtypes

import concourse.bass as bass
import concourse.mybir as mybir
import concourse.tile as tile
from concourse import bacc
from concourse.bass import ts
from concourse.bass_utils import run_bass_kernel_spmd

B, N, D = 4, 1024, 128
NCORES = 8
NT = 512            # tokens retrieved per core (half a batch)
TT = 512            # store-phase token tile
NTI = N // TT
NCHUNK = N // 128
RH = 256            # retrieval sub-tile

# bf16 weight packs:
#  wpbu (urgent): w0eff=Wk@w0 | w1 | w2 | w3s=(2/D)w3 | wv_r=-(2/D)Wv
#                 | wq | wkq_t=wq@Wk^T
#  wpbr (rest):   w1^T | w2^T | w3^T | w0 | ident

f32 = mybir.dt.float32
bf16 = mybir.dt.bfloat16

AF = mybir.ActivationFunctionType
ALU = mybir.AluOpType


def _build_program(reps=1):
    nc = bacc.Bacc(
        "TRN2",
        target_bir_lowering=False,
        debug=False,
        enable_asserts=False,
        num_devices=NCORES,
    )

    st_dr = nc.dram_tensor("st", [128, N], bf16, kind="ExternalInput").ap()
    stm_dr = nc.dram_tensor("s_tmb", [128, N], bf16, kind="ExternalInput").ap()
    we_dr = nc.dram_tensor("w0eff", [D, D], bf16, kind="ExternalInput").ap()
    wbu_dr = nc.dram_tensor("wpbu", [D, 6 * D], bf16, kind="ExternalInput").ap()
    wbr_dr = nc.dram_tensor("wpbr", [D, 5 * D], bf16, kind="ExternalInput").ap()
    wf_dr = nc.dram_tensor("wpf", [D, 3 * D], f32, kind="ExternalInput").ap()
    out_dr = nc.dram_tensor("out", [NT, D], bf16, kind="ExternalOutput").ap()

    with tile.TileContext(nc) as tc:
        for _ in range(reps):
            _emit(tc, st_dr, stm_dr, we_dr, wbu_dr, wbr_dr, wf_dr, out_dr)

    nc.compile()
    return nc


def _emit(tc, st_dr, stm_dr, we_dr, wbu_dr, wbr_dr, wf_dr, out_dr):
    nc = tc.nc
    from contextlib import ExitStack

    from concourse.tile_rust import add_dep_helper as _dep  # type: ignore

    with ExitStack() as ctx:
        consts = ctx.enter_context(tc.tile_pool(name="consts", bufs=1))
        big = ctx.enter_context(tc.tile_pool(name="big", bufs=1))
        # PSUM: ha0,ha1,hb0,hb1,hc0,hc1 (1 bank each) + st(2) = 8 banks
        pp = ctx.enter_context(tc.tile_pool(name="pp", bufs=1, space="PSUM"))

        def pbank(tag, name, shape=None, dt=f32):
            return pp.tile(shape or [128, TT], dt, tag=tag, bufs=1, name=name)

        def pstage(name, w=512, dt=f32):
            return pp.tile([128, w], dt, tag="stg", bufs=2, name=name)

        # ---- DMAs, ordered by need (all HWDGE on the sync queue);
        # w0eff ships alone (32KB) so layer 0 starts right after S^T ----
        w0eff_t = consts.tile([D, D], bf16, tag="w0eff")
        wpbu = consts.tile([D, 6 * D], bf16, tag="wpbu")
        wpbr = consts.tile([D, 5 * D], bf16, tag="wpbr")
        stt = big.tile([128, N], bf16, tag="stt")      # S^T feature-major
        s_tmb = big.tile([128, NCHUNK, 128], bf16, tag="s_tmb")  # token-major
        wpf = consts.tile([D, 3 * D], f32, tag="wpf")
        nc.sync.dma_start(w0eff_t[:], we_dr)
        nc.sync.dma_start(stt[:], st_dr)
        nc.sync.dma_start(wpbu[:], wbu_dr)
        nc.sync.dma_start(wpbr[:], wbr_dr)
        nc.sync.dma_start(
            s_tmb[:], stm_dr.rearrange("p (c d) -> p c d", d=128)
        )
        nc.sync.dma_start(wpf[:], wf_dr)

        w0eff = w0eff_t[:]
        w1b = wpbu[:, ts(0, D)]
        w2b = wpbu[:, ts(1, D)]
        w3s = wpbu[:, ts(2, D)]
        wv_r = wpbu[:, ts(3, D)]
        wqb = wpbu[:, ts(4, D)]
        wkq_t = wpbu[:, ts(5, D)]                      # wq @ Wk^T
        wt = [wpbr[:, ts(i, D)] for i in range(3)]     # w1^T,w2^T,w3^T
        w0b = wpbr[:, ts(3, D)]
        ident_b = wpbr[:, ts(4, D)]
        w_f = [wpf[:, ts(i, D)] for i in range(3)]     # w1,w2,w3 f32

        # tiny scratch silu pulls the first ACT table load off the
        # critical path (runs during the DMAs)
        scr = consts.tile([128, 1], f32, tag="scr")
        scr2 = consts.tile([128, 1], f32, tag="scr2")
        nc.gpsimd.memset(scr[:], 0.0)
        nc.scalar.activation(scr2[:], scr[:], AF.Silu)

        # PE warm-up: start the HAM clock window early so real matmuls
        # hit full clock by ~3us
        wupa = consts.tile([128, 128], f32, tag="wupa")
        nc.gpsimd.memset(wupa[:], 0.0)
        wupp = pstage("wupp")
        for _ in range(6):
            nc.tensor.matmul(
                wupp[:, 0:128], wupa[:], wupa[:],
                skip_group_check=True,
            )

        # per-half persistent SBUF tensors (feature-major, bf16)
        def halves(pfx):
            return [
                big.tile([128, TT], bf16, name=f"{pfx}{t}", tag=f"{pfx}{t}")
                for t in range(NTI)
            ]

        a1 = halves("a1")
        a2 = halves("a2")
        a3 = halves("a3")
        sp0 = halves("sp0")
        sp1 = halves("sp1")
        sp2 = halves("sp2")
        g0 = halves("g0")
        g1 = halves("g1")
        g2 = halves("g2")
        g3 = halves("g3")   # (2/D)(H3 - V)

        # ---- x0 = wq^T S^T and pt = (wq Wk^T)^T S^T: two independent
        # matmuls straight off S^T (host pre-composed wkq_t = wq @ Wk^T),
        # evicted on DVE long before the backward muls need it ----
        x0 = big.tile([128, NT], bf16, tag="x0")
        px = pstage("p_x0")
        nc.tensor.matmul(px[:], wqb, stt[:, 0:NT])
        nc.vector.tensor_copy(x0[:], px[:])
        pt = big.tile([128, NT], bf16, tag="pt")
        px = pstage("p_pt")
        nc.tensor.matmul(px[:], wkq_t, stt[:, 0:NT])
        nc.vector.tensor_copy(pt[:], px[:])

        silu_insts = []
        dsilu_insts = []

        # ---- forward: six single-bank H tiles; silu -> bf16 halves ----
        hb = {}
        for li, tag in ((0, "ha"), (1, "hb"), (2, "hc")):
            for t in range(NTI):
                hb[li, t] = pbank(f"{tag}{t}", f"h{li}_{t}")
        for t in range(NTI):
            sl = ts(t, TT)
            nc.tensor.matmul(hb[0, t][:], w0eff, stt[:, sl])
            silu_insts.append(nc.scalar.activation(a1[t][:], hb[0, t][:], AF.Silu))
            nc.tensor.matmul(hb[1, t][:], w1b, a1[t][:])
            silu_insts.append(nc.scalar.activation(a2[t][:], hb[1, t][:], AF.Silu))
            nc.tensor.matmul(hb[2, t][:], w2b, a2[t][:])
            silu_insts.append(nc.scalar.activation(a3[t][:], hb[2, t][:], AF.Silu))
            # H3 - V accumulated in one stage bank (wv_r is negated+scaled)
            h3 = pstage(f"h3_{t}")
            nc.tensor.matmul(h3[:], wv_r, stt[:, sl], start=True, stop=False)
            nc.tensor.matmul(h3[:], w3s, a3[t][:], start=False, stop=True)
            nc.vector.tensor_copy(g3[t][:], h3[:])

        # ---- dsilu after all silus (one table switch), consumption order --
        for spd, li in ((sp2, 2), (sp1, 1), (sp0, 0)):
            for t in (1, 0):
                di = nc.scalar.activation(
                    spd[t][:], hb[li, t][:], AF.Derivative_silu
                )
                dsilu_insts.append(di)
        for di in dsilu_insts:
            _dep(di.ins, silu_insts[-1].ins, sync=False, reason="act-table order")

        # ---- backward: tile-1 first (it gates M); c tiles reuse the H
        # banks (freed in dsilu order, which matches mul consumption) ----
        cb = {}
        for li, tag in ((2, "hc"), (1, "hb"), (0, "ha")):
            for t in (1, 0):
                cb[li, t] = pbank(f"{tag}{t}", f"c{li}_{t}")
        for li, gin, gout, spd in (
            (2, g3, g2, sp2), (1, g2, g1, sp1), (0, g1, g0, sp0)
        ):
            for t in (1, 0):
                nc.tensor.matmul(cb[li, t][:], wt[li], gin[t][:])
                nc.vector.tensor_mul(gout[t][:], cb[li, t][:], spd[t][:])

        # reload the silu table during the dW phase, off the tail
        scr3 = consts.tile([128, 1], f32, tag="scr3")
        dummy = nc.scalar.activation(scr3[:], scr[:], AF.Silu)
        _dep(dummy.ins, dsilu_insts[-1].ins, sync=False, reason="act-table order")

        # ---- token-major copies: XBAR for a1..a3,g3..g1; PE path for g0 ----
        a_tm = [None] + [
            big.tile([128, NCHUNK, 128], bf16, name=f"atm{i}", tag=f"atm{i}")
            for i in (1, 2, 3)
        ]
        g_tm = [
            big.tile([128, NCHUNK, 128], bf16, name=f"gtm{i}", tag=f"gtm{i}")
            for i in range(4)
        ]
        for src, dst in (
            (a1, a_tm[1]), (a2, a_tm[2]), (a3, a_tm[3]),
            (g3, g_tm[3]), (g2, g_tm[2]), (g1, g_tm[1]),
        ):
            for h in range(NTI):
                nc.sync.dma_start_transpose(dst[:, 4 * h : 4 * h + 4], src[h][:])

        # g0: PE transposes + DVE half evicts (lowest latency on the
        # tail); separate stage tiles per half so the h0 transposes don't
        # wait on the h1 eviction (tile-granular WAR). Both g0 muls are
        # emitted before the evicts so DVE drains the muls first.
        pg0 = [pstage(f"p_g0{h}", w=512, dt=bf16) for h in range(NTI)]
        for h in (1, 0):
            for j in range(4):
                nc.tensor.matmul(
                    pg0[h][:, ts(j, 128)], g0[h][:, ts(j, 128)], ident_b,
                    is_transpose=True,
                )
        # h1 evict on ACT (idle during the dW phase); h0 on DVE -- they
        # drain in parallel instead of serializing on DVE
        nc.scalar.activation(
            g_tm[0][:, 4:8].rearrange("p c d -> p (c d)"), pg0[1][:], AF.Copy
        )
        nc.vector.tensor_copy(
            g_tm[0][:, 0:4], pg0[0][:].rearrange("p (c d) -> p c d", d=128)
        )

        # ---- M = S^T G0 in its OWN bank/group so m_r never waits the
        # late dW1 xbar round-trip ----
        macc = pstage("macc", w=128)
        m_stop = None
        for k, c in enumerate((4, 5, 6, 7, 0, 1, 2, 3)):
            m_stop = nc.tensor.matmul(
                macc[:, 0:128],
                s_tmb[:, c],
                g_tm[0][:, c],
                start=(k == 0),
                stop=(c == 3),
            )
        m_r = big.tile([128, 128], bf16, tag="m_r")
        # ACT is idle here; DVE is still draining the g0 evicts
        nc.scalar.activation(m_r[:], macc[:, 0:128], AF.Copy)

        # ---- dW3/dW2/dW1 in one PSUM accumulation group ----
        acc = pbank("hc1", "dwacc", shape=[128, 3, 128])
        dw_stop = None
        for k, (i, slot) in enumerate(((3, 0), (2, 1), (1, 2))):
            for c in range(NCHUNK):
                dw_stop = nc.tensor.matmul(
                    acc[:, slot],
                    a_tm[i][:, c],
                    g_tm[i][:, c],
                    start=(k == 0 and c == 0),
                    stop=(slot == 2 and c == NCHUNK - 1),
                )

        u = [None]
        for slot, i in ((2, 0), (1, 1), (0, 2)):
            ut = big.tile([D, D], bf16, name=f"u{i}", tag=f"u{i}")
            ai = nc.vector.tensor_add(ut[:], acc[:, slot], w_f[i])
            # same-bank safety: no reads before the group's stop matmul
            _dep(ai.ins, dw_stop.ins, sync=True, reason="acc bank group")
            u.append(ut)
        # u[1]=w1+dW1, u[2]=w2+dW2, u[3]=w3+dW3

        # ---- retrieval: X1 = X0 @ w0 + P @ M, then layers 2..4 ------------
        # per-half tiles throughout so the two half-pipelines don't
        # serialize on tile-granular deps
        r1, r2, r3 = [], [], []
        for h in range(NTI):
            r1.append(big.tile([128, RH], bf16, name=f"r1h{h}", tag=f"r1h{h}"))
            r2.append(big.tile([128, RH], bf16, name=f"r2h{h}", tag=f"r2h{h}"))
            r3.append(big.tile([128, RH], bf16, name=f"r3h{h}", tag=f"r3h{h}"))

        nh = NT // RH
        px1 = [pbank(f"ha{hh}", f"px1_{hh}", shape=[128, RH]) for hh in range(nh)]
        for hh in range(nh):
            # term 1 (X0 @ w0) has no M dependency -- runs during the dW phase
            nc.tensor.matmul(
                px1[hh][:], w0b, x0[:, ts(hh, RH)], start=True, stop=False
            )
        for hh in range(nh):
            nc.tensor.matmul(
                px1[hh][:], m_r[:], pt[:, ts(hh, RH)], start=False, stop=True
            )
            nc.scalar.activation(r1[hh][:], px1[hh][:], AF.Silu)
        px2 = [pbank(f"hb{hh}", f"px2_{hh}", shape=[128, RH]) for hh in range(nh)]
        for hh in range(nh):
            nc.tensor.matmul(px2[hh][:], u[1][:], r1[hh][:])
            nc.scalar.activation(r2[hh][:], px2[hh][:], AF.Silu)
        px3 = [
            pbank("hc0", "px3_0", shape=[128, RH]),
            pstage("px3_1", w=RH),
        ]
        for hh in range(nh):
            nc.tensor.matmul(px3[hh][:], u[2][:], r2[hh][:])
            nc.scalar.activation(r3[hh][:], px3[hh][:], AF.Silu)
        out_r = out_dr.rearrange("(c p) d -> p c d", p=128)
        for hh in range(nh):
            po = pstage(f"po{hh}", w=RH)
            pov = po[:].rearrange("p (c d) -> p c d", d=128)
            for j in range(RH // 128):
                nc.tensor.matmul(
                    pov[:, j], r3[hh][:, ts(j, 128)], u[3][:],
                    start=(j == 0), stop=(j == RH // 128 - 1),
                )
            o_tm = big.tile([128, 2, 128], bf16, name=f"o_tm{hh}", tag=f"o_tm{hh}")
            nc.vector.tensor_copy(o_tm[:], pov[:])
            nc.sync.dma_start(out_r[:, 2 * hh : 2 * hh + 2], o_tm[:])


_CACHE = {}


def _get_nc():
    if "nc" not in _CACHE:
        _CACHE["nc"] = _build_program()
    return _CACHE["nc"]


def _bf(x):
    return np.ascontiguousarray(x.astype(ml_dtypes.bfloat16))


def _prep_weights(w0, w1, w2, w3, wq, wkv):
    """Host-side weight-space prep (layout, transposes, scales, composes)."""
    w0, w1, w2, w3, wq, wkv = (
        np.asarray(x, np.float32) for x in (w0, w1, w2, w3, wq, wkv)
    )
    wk, wv = wkv[:, :D], wkv[:, D:]
    ident = np.eye(D, dtype=np.float32)
    w0eff = wk @ w0
    wpbu = np.concatenate(
        [
            w1, w2,
            (2.0 / D) * w3,     # w3s
            (-2.0 / D) * wv,    # wv_r
            wq,                 # wqb
            wq @ wk.T,          # wkq_t: pt = (wq Wk^T)^T S^T
        ],
        axis=1,
    )
    wpbr = np.concatenate([w1.T, w2.T, w3.T, w0, ident], axis=1)
    wpf = np.ascontiguousarray(np.concatenate([w1, w2, w3], axis=1))
    return _bf(w0eff), _bf(wpbu), _bf(wpbr), wpf


def kernel(seq, w0, w1, w2, w3, wq, wkv):
    nc = _get_nc()
    seq = np.asarray(seq, np.float32)
    w0eff, wpbu, wpbr, wpf = _prep_weights(w0, w1, w2, w3, wq, wkv)

    in_maps = []
    for c in range(NCORES):
        b, h = c // 2, c % 2
        if h == 0:
            s = seq[b]
        else:
            # rotate: retrieval half first; grad sum is order-invariant
            s = np.concatenate([seq[b, NT:], seq[b, :NT]], axis=0)
        sb = s.astype(ml_dtypes.bfloat16)
        # token-major [128, c, d] flattened: partition p, token c*128+p
        stm = np.ascontiguousarray(
            sb.reshape(NCHUNK, 128, D).transpose(1, 0, 2).reshape(128, N)
        )
        in_maps.append(
            {
                "st": np.ascontiguousarray(sb.T),
                "s_tmb": stm,
                "w0eff": w0eff,
                "wpbu": wpbu,
                "wpbr": wpbr,
                "wpf": wpf,
            }
        )

    res = run_bass_kernel_spmd(nc, in_maps, core_ids=list(range(NCORES)))
    _CACHE["last_results"] = res

    out = np.empty((B, N, D), np.float32)
    for c in range(NCORES):
        b, h = c // 2, c % 2
        out[b, h * NT : (h + 1) * NT] = res.results[c]["out"].astype(np.float32)
    return out


# revision 24
# speedup vs baseline: 1.3558x; 1.0072x over previous
"""Trainium2 Bass kernel for nn_NeuralMemory (B=4, N=1024, D=128, DEPTH=4).

Sharding: 8 cores, core c handles batch b = c//2. Both cores of a pair
compute the store phase (per-token grads summed over all 1024 tokens)
redundantly -- the grad sum is order-invariant, so each core gets its
batch's sequence with its own retrieval half rotated to the front and
retrieves tokens [0:512) of its view. No collectives (a pair AllReduce
has a ~10us floor, worse than the duplicated compute).

v4 design notes:
  - All on-chip tensors are bf16 except f32 PSUM accumulators, so every
    producer writes bf16 directly (no cast chain).
  - Host-side prep (layout/weight-space only, no token-dim compute):
    seq is shipped twice in bf16 -- token-major (s_tmb, for M = S^T G0)
    and feature-major (st = S^T, for the forward); the [d,d] weights
    ship pre-transposed/pre-scaled/pre-composed in bf16 packs (Wk@w0
    alone in a 32KB first DMA so layer 0 starts right after S^T lands;
    wq@Wk^T so x0 and pt are independent matmuls off S^T; w^T's,
    +-(2/D) scales, identity) plus a small f32 pack for the
    u_i = w_i + dW_i adds.
  - Dependency tracking is tile-granular, so every per-half-written or
    per-half-read tensor is split into separate tiles (a_i, g_i, sp_i,
    c_i, H_i, px_i, r_i, o_tm halves) -- otherwise write-after-read
    false deps serialize the two pipelines.
  - forward Silu / backward Derivative_silu read the f32 H PSUM banks
    directly; H0..H2 live in six single-bank tiles whose banks are
    reused by the backward c tiles, then the retrieval px tiles.
  - token-major copies for the dW contraction run on the DMA XBAR
    (dma_start_transpose, SBUF->SBUF bf16, ~450ns per [128,512] half on
    otherwise-idle DMA engines); only g0 (which gates M on the critical
    tail) keeps the lower-latency PE-transpose + DVE-evict path.
  - backward runs tile-1-first layer-major; M = S^T G0 accumulates in
    its own PSUM bank (so m_r never waits the dW group's late xbar
    inputs); dW3/dW2/dW1 share a second accumulation group feeding the
    u_i adds. Retrieval is X1 = X0 w0 + (X0 Wk^T) M with the X0 w0 term
    pre-accumulated, so only the tiny M eviction sits on the critical
    tail; the output ships in the device's token-chunk layout and the
    host reassembles.
  - ACT-table discipline: all Silus, then all Derivative_silus, then a
    dummy Silu reload during the dW phase.
"""

import numpy as np
import ml_dtypes

import concourse.bass as bass
import concourse.mybir as mybir
import concourse.tile as tile
from concourse import bacc
from concourse.bass import ts
from concourse.bass_utils import run_bass_kernel_spmd

B, N, D = 4, 1024, 128
NCORES = 8
NT = 512            # tokens retrieved per core (half a batch)
TT = 512            # store-phase token tile
NTI = N // TT
NCHUNK = N // 128
RH = 256            # retrieval sub-tile

# bf16 weight packs:
#  wpbu (urgent): w0eff=Wk@w0 | w1 | w2 | w3s=(2/D)w3 | wv_r=-(2/D)Wv
#                 | wq | wkq_t=wq@Wk^T
#  wpbr (rest):   w1^T | w2^T | w3^T | w0 | ident

f32 = mybir.dt.float32
bf16 = mybir.dt.bfloat16

AF = mybir.ActivationFunctionType
ALU = mybir.AluOpType


def _build_program(reps=1):
    nc = bacc.Bacc(
        "TRN2",
        target_bir_lowering=False,
        debug=False,
        enable_asserts=False,
        num_devices=NCORES,
    )

    st_dr = nc.dram_tensor("st", [128, N], bf16, kind="ExternalInput").ap()
    stm_dr = nc.dram_tensor("s_tmb", [128, N], bf16, kind="ExternalInput").ap()
    we_dr = nc.dram_tensor("w0eff", [D, D], bf16, kind="ExternalInput").ap()
    wbu_dr = nc.dram_tensor("wpbu", [D, 6 * D], bf16, kind="ExternalInput").ap()
    wbr_dr = nc.dram_tensor("wpbr", [D, 5 * D], bf16, kind="ExternalInput").ap()
    wf_dr = nc.dram_tensor("wpf", [D, 3 * D], f32, kind="ExternalInput").ap()
    out_dr = nc.dram_tensor("out", [128, NT // 128, D], bf16, kind="ExternalOutput").ap()

    with tile.TileContext(nc) as tc:
        for _ in range(reps):
            _emit(tc, st_dr, stm_dr, we_dr, wbu_dr, wbr_dr, wf_dr, out_dr)

    nc.compile()
    return nc


def _emit(tc, st_dr, stm_dr, we_dr, wbu_dr, wbr_dr, wf_dr, out_dr):
    nc = tc.nc
    from contextlib import ExitStack

    from concourse.tile_rust import add_dep_helper as _dep  # type: ignore

    with ExitStack() as ctx:
        consts = ctx.enter_context(tc.tile_pool(name="consts", bufs=1))
        big = ctx.enter_context(tc.tile_pool(name="big", bufs=1))
        # PSUM: ha0,ha1,hb0,hb1,hc0,hc1 (1 bank each) + st(2) = 8 banks
        pp = ctx.enter_context(tc.tile_pool(name="pp", bufs=1, space="PSUM"))

        def pbank(tag, name, shape=None, dt=f32):
            return pp.tile(shape or [128, TT], dt, tag=tag, bufs=1, name=name)

        def pstage(name, w=512, dt=f32):
            return pp.tile([128, w], dt, tag="stg", bufs=2, name=name)

        # ---- DMAs, ordered by need (all HWDGE on the sync queue);
        # w0eff ships alone (32KB) so layer 0 starts right after S^T ----
        w0eff_t = consts.tile([D, D], bf16, tag="w0eff")
        wpbu = consts.tile([D, 6 * D], bf16, tag="wpbu")
        wpbr = consts.tile([D, 5 * D], bf16, tag="wpbr")
        stt = big.tile([128, N], bf16, tag="stt")      # S^T feature-major
        s_tmb = big.tile([128, NCHUNK, 128], bf16, tag="s_tmb")  # token-major
        wpf = consts.tile([D, 3 * D], f32, tag="wpf")
        nc.sync.dma_start(w0eff_t[:], we_dr)
        nc.sync.dma_start(stt[:], st_dr)
        nc.sync.dma_start(wpbu[:], wbu_dr)
        nc.sync.dma_start(wpbr[:], wbr_dr)
        nc.sync.dma_start(
            s_tmb[:], stm_dr.rearrange("p (c d) -> p c d", d=128)
        )
        nc.sync.dma_start(wpf[:], wf_dr)

        w0eff = w0eff_t[:]
        w1b = wpbu[:, ts(0, D)]
        w2b = wpbu[:, ts(1, D)]
        w3s = wpbu[:, ts(2, D)]
        wv_r = wpbu[:, ts(3, D)]
        wqb = wpbu[:, ts(4, D)]
        wkq_t = wpbu[:, ts(5, D)]                      # wq @ Wk^T
        wt = [wpbr[:, ts(i, D)] for i in range(3)]     # w1^T,w2^T,w3^T
        w0b = wpbr[:, ts(3, D)]
        ident_b = wpbr[:, ts(4, D)]
        w_f = [wpf[:, ts(i, D)] for i in range(3)]     # w1,w2,w3 f32

        # tiny scratch silu pulls the first ACT table load off the
        # critical path (runs during the DMAs)
        scr = consts.tile([128, 1], f32, tag="scr")
        scr2 = consts.tile([128, 1], f32, tag="scr2")
        nc.gpsimd.memset(scr[:], 0.0)
        nc.scalar.activation(scr2[:], scr[:], AF.Silu)

        # PE warm-up: start the HAM clock window early so real matmuls
        # hit full clock by ~3us
        wupa = consts.tile([128, 128], f32, tag="wupa")
        nc.gpsimd.memset(wupa[:], 0.0)
        wupp = pstage("wupp")
        for _ in range(6):
            nc.tensor.matmul(
                wupp[:, 0:128], wupa[:], wupa[:],
                skip_group_check=True,
            )

        # per-half persistent SBUF tensors (feature-major, bf16)
        def halves(pfx):
            return [
                big.tile([128, TT], bf16, name=f"{pfx}{t}", tag=f"{pfx}{t}")
                for t in range(NTI)
            ]

        a1 = halves("a1")
        a2 = halves("a2")
        a3 = halves("a3")
        sp0 = halves("sp0")
        sp1 = halves("sp1")
        sp2 = halves("sp2")
        g0 = halves("g0")
        g1 = halves("g1")
        g2 = halves("g2")
        g3 = halves("g3")   # (2/D)(H3 - V)

        # ---- x0 = wq^T S^T and pt = (wq Wk^T)^T S^T: two independent
        # matmuls straight off S^T (host pre-composed wkq_t = wq @ Wk^T),
        # evicted on DVE long before the backward muls need it ----
        x0 = big.tile([128, NT], bf16, tag="x0")
        px = pstage("p_x0")
        nc.tensor.matmul(px[:], wqb, stt[:, 0:NT])
        nc.vector.tensor_copy(x0[:], px[:])
        pt = big.tile([128, NT], bf16, tag="pt")
        px = pstage("p_pt")
        nc.tensor.matmul(px[:], wkq_t, stt[:, 0:NT])
        nc.vector.tensor_copy(pt[:], px[:])

        silu_insts = []
        dsilu_insts = []

        # ---- forward: six single-bank H tiles; silu -> bf16 halves ----
        hb = {}
        for li, tag in ((0, "ha"), (1, "hb"), (2, "hc")):
            for t in range(NTI):
                hb[li, t] = pbank(f"{tag}{t}", f"h{li}_{t}")
        for t in range(NTI):
            sl = ts(t, TT)
            nc.tensor.matmul(hb[0, t][:], w0eff, stt[:, sl])
            silu_insts.append(nc.scalar.activation(a1[t][:], hb[0, t][:], AF.Silu))
            nc.tensor.matmul(hb[1, t][:], w1b, a1[t][:])
            silu_insts.append(nc.scalar.activation(a2[t][:], hb[1, t][:], AF.Silu))
            nc.tensor.matmul(hb[2, t][:], w2b, a2[t][:])
            silu_insts.append(nc.scalar.activation(a3[t][:], hb[2, t][:], AF.Silu))
            # H3 - V accumulated in one stage bank (wv_r is negated+scaled)
            h3 = pstage(f"h3_{t}")
            nc.tensor.matmul(h3[:], wv_r, stt[:, sl], start=True, stop=False)
            nc.tensor.matmul(h3[:], w3s, a3[t][:], start=False, stop=True)
            nc.vector.tensor_copy(g3[t][:], h3[:])

        # ---- dsilu after all silus (one table switch), consumption order --
        for spd, li in ((sp2, 2), (sp1, 1), (sp0, 0)):
            for t in (1, 0):
                di = nc.scalar.activation(
                    spd[t][:], hb[li, t][:], AF.Derivative_silu
                )
                dsilu_insts.append(di)
        for di in dsilu_insts:
            _dep(di.ins, silu_insts[-1].ins, sync=False, reason="act-table order")

        # ---- backward: tile-1 first (it gates M); c tiles reuse the H
        # banks (freed in dsilu order, which matches mul consumption) ----
        cb = {}
        for li, tag in ((2, "hc"), (1, "hb"), (0, "ha")):
            for t in (1, 0):
                cb[li, t] = pbank(f"{tag}{t}", f"c{li}_{t}")
        for li, gin, gout, spd in (
            (2, g3, g2, sp2), (1, g2, g1, sp1), (0, g1, g0, sp0)
        ):
            for t in (1, 0):
                nc.tensor.matmul(cb[li, t][:], wt[li], gin[t][:])
                nc.vector.tensor_mul(gout[t][:], cb[li, t][:], spd[t][:])

        # reload the silu table during the dW phase, off the tail
        scr3 = consts.tile([128, 1], f32, tag="scr3")
        dummy = nc.scalar.activation(scr3[:], scr[:], AF.Silu)
        _dep(dummy.ins, dsilu_insts[-1].ins, sync=False, reason="act-table order")

        # ---- token-major copies: XBAR for a1..a3,g3..g1; PE path for g0 ----
        a_tm = [None] + [
            big.tile([128, NCHUNK, 128], bf16, name=f"atm{i}", tag=f"atm{i}")
            for i in (1, 2, 3)
        ]
        g_tm = [
            big.tile([128, NCHUNK, 128], bf16, name=f"gtm{i}", tag=f"gtm{i}")
            for i in range(4)
        ]
        for src, dst in (
            (a1, a_tm[1]), (a2, a_tm[2]), (a3, a_tm[3]),
            (g3, g_tm[3]), (g2, g_tm[2]), (g1, g_tm[1]),
        ):
            for h in range(NTI):
                nc.sync.dma_start_transpose(dst[:, 4 * h : 4 * h + 4], src[h][:])

        # g0: PE transposes + DVE half evicts (lowest latency on the
        # tail); separate stage tiles per half so the h0 transposes don't
        # wait on the h1 eviction (tile-granular WAR). Both g0 muls are
        # emitted before the evicts so DVE drains the muls first.
        pg0 = [pstage(f"p_g0{h}", w=512, dt=bf16) for h in range(NTI)]
        for h in (1, 0):
            for j in range(4):
                nc.tensor.matmul(
                    pg0[h][:, ts(j, 128)], g0[h][:, ts(j, 128)], ident_b,
                    is_transpose=True,
                )
        # h1 evict on ACT (idle during the dW phase); h0 on DVE -- they
        # drain in parallel instead of serializing on DVE
        nc.scalar.activation(
            g_tm[0][:, 4:8].rearrange("p c d -> p (c d)"), pg0[1][:], AF.Copy
        )
        nc.vector.tensor_copy(
            g_tm[0][:, 0:4], pg0[0][:].rearrange("p (c d) -> p c d", d=128)
        )

        # ---- M = S^T G0 in its OWN bank/group so m_r never waits the
        # late dW1 xbar round-trip ----
        macc = pstage("macc", w=128)
        m_stop = None
        for k, c in enumerate((4, 5, 6, 7, 0, 1, 2, 3)):
            m_stop = nc.tensor.matmul(
                macc[:, 0:128],
                s_tmb[:, c],
                g_tm[0][:, c],
                start=(k == 0),
                stop=(c == 3),
            )
        m_r = big.tile([128, 128], bf16, tag="m_r")
        # ACT is idle here; DVE is still draining the g0 evicts
        nc.scalar.activation(m_r[:], macc[:, 0:128], AF.Copy)

        # ---- dW3/dW2/dW1 in one PSUM accumulation group ----
        acc = pbank("hc1", "dwacc", shape=[128, 3, 128])
        dw_stop = None
        for k, (i, slot) in enumerate(((3, 0), (2, 1), (1, 2))):
            for c in range(NCHUNK):
                dw_stop = nc.tensor.matmul(
                    acc[:, slot],
                    a_tm[i][:, c],
                    g_tm[i][:, c],
                    start=(k == 0 and c == 0),
                    stop=(slot == 2 and c == NCHUNK - 1),
                )

        u = [None]
        for slot, i in ((2, 0), (1, 1), (0, 2)):
            ut = big.tile([D, D], bf16, name=f"u{i}", tag=f"u{i}")
            ai = nc.vector.tensor_add(ut[:], acc[:, slot], w_f[i])
            # same-bank safety: no reads before the group's stop matmul
            _dep(ai.ins, dw_stop.ins, sync=True, reason="acc bank group")
            u.append(ut)
        # u[1]=w1+dW1, u[2]=w2+dW2, u[3]=w3+dW3

        # ---- retrieval: X1 = X0 @ w0 + P @ M, then layers 2..4 ------------
        # per-half tiles throughout so the two half-pipelines don't
        # serialize on tile-granular deps
        r1, r2, r3 = [], [], []
        for h in range(NTI):
            r1.append(big.tile([128, RH], bf16, name=f"r1h{h}", tag=f"r1h{h}"))
            r2.append(big.tile([128, RH], bf16, name=f"r2h{h}", tag=f"r2h{h}"))
            r3.append(big.tile([128, RH], bf16, name=f"r3h{h}", tag=f"r3h{h}"))

        nh = NT // RH
        px1 = [pbank(f"ha{hh}", f"px1_{hh}", shape=[128, RH]) for hh in range(nh)]
        for hh in range(nh):
            # term 1 (X0 @ w0) has no M dependency -- runs during the dW phase
            nc.tensor.matmul(
                px1[hh][:], w0b, x0[:, ts(hh, RH)], start=True, stop=False
            )
        for hh in range(nh):
            nc.tensor.matmul(
                px1[hh][:], m_r[:], pt[:, ts(hh, RH)], start=False, stop=True
            )
            nc.scalar.activation(r1[hh][:], px1[hh][:], AF.Silu)
        px2 = [pbank(f"hb{hh}", f"px2_{hh}", shape=[128, RH]) for hh in range(nh)]
        for hh in range(nh):
            nc.tensor.matmul(px2[hh][:], u[1][:], r1[hh][:])
            nc.scalar.activation(r2[hh][:], px2[hh][:], AF.Silu)
        px3 = [
            pbank("hc0", "px3_0", shape=[128, RH]),
            pstage("px3_1", w=RH),
        ]
        for hh in range(nh):
            nc.tensor.matmul(px3[hh][:], u[2][:], r2[hh][:])
            nc.scalar.activation(r3[hh][:], px3[hh][:], AF.Silu)
        out_r = out_dr  # [p, c, d]: token c*128+p, contiguous per partition
        for hh in range(nh):
            po = pstage(f"po{hh}", w=RH)
            pov = po[:].rearrange("p (c d) -> p c d", d=128)
            for j in range(RH // 128):
                nc.tensor.matmul(
                    pov[:, j], r3[hh][:, ts(j, 128)], u[3][:],
                    start=(j == 0), stop=(j == RH // 128 - 1),
                )
            o_tm = big.tile([128, 2, 128], bf16, name=f"o_tm{hh}", tag=f"o_tm{hh}")
            nc.vector.tensor_copy(o_tm[:], pov[:])
            nc.sync.dma_start(out_r[:, 2 * hh : 2 * hh + 2], o_tm[:])


_CACHE = {}


def _get_nc():
    if "nc" not in _CACHE:
        _CACHE["nc"] = _build_program()
    return _CACHE["nc"]


def _bf(x):
    return np.ascontiguousarray(x.astype(ml_dtypes.bfloat16))


def _prep_weights(w0, w1, w2, w3, wq, wkv):
    """Host-side weight-space prep (layout, transposes, scales, composes)."""
    w0, w1, w2, w3, wq, wkv = (
        np.asarray(x, np.float32) for x in (w0, w1, w2, w3, wq, wkv)
    )
    wk, wv = wkv[:, :D], wkv[:, D:]
    ident = np.eye(D, dtype=np.float32)
    w0eff = wk @ w0
    wpbu = np.concatenate(
        [
            w1, w2,
            (2.0 / D) * w3,     # w3s
            (-2.0 / D) * wv,    # wv_r
            wq,                 # wqb
            wq @ wk.T,          # wkq_t: pt = (wq Wk^T)^T S^T
        ],
        axis=1,
    )
    wpbr = np.concatenate([w1.T, w2.T, w3.T, w0, ident], axis=1)
    wpf = np.ascontiguousarray(np.concatenate([w1, w2, w3], axis=1))
    return _bf(w0eff), _bf(wpbu), _bf(wpbr), wpf


def kernel(seq, w0, w1, w2, w3, wq, wkv):
    nc = _get_nc()
    seq = np.asarray(seq, np.float32)
    w0eff, wpbu, wpbr, wpf = _prep_weights(w0, w1, w2, w3, wq, wkv)

    in_maps = []
    for c in range(NCORES):
        b, h = c // 2, c % 2
        if h == 0:
            s = seq[b]
        else:
            # rotate: retrieval half first; grad sum is order-invariant
            s = np.concatenate([seq[b, NT:], seq[b, :NT]], axis=0)
        sb = s.astype(ml_dtypes.bfloat16)
        # token-major [128, c, d] flattened: partition p, token c*128+p
        stm = np.ascontiguousarray(
            sb.reshape(NCHUNK, 128, D).transpose(1, 0, 2).reshape(128, N)
        )
        in_maps.append(
            {
                "st": np.ascontiguousarray(sb.T),
                "s_tmb": stm,
                "w0eff": w0eff,
                "wpbu": wpbu,
                "wpbr": wpbr,
                "wpf": wpf,
            }
        )

    res = run_bass_kernel_spmd(nc, in_maps, core_ids=list(range(NCORES)))
    _CACHE["last_results"] = res

    out = np.empty((B, N, D), np.float32)
    for c in range(NCORES):
        b, h = c // 2, c % 2
        # device layout [p, chunk, d] -> tokens (chunk*128+p, d)
        ob = res.results[c]["out"].astype(np.float32)
        out[b, h * NT : (h + 1) * NT] = ob.transpose(1, 0, 2).reshape(NT, D)
    return out
